# revision 7
# baseline (speedup 1.0000x reference)
"""Trainium2 Bass kernel: single-head causal attention (B=8, T=2048, D=1024, HS=64).

Sharding: data-parallel over batch B -- one batch element per NeuronCore (8 cores).
Host-side prep (part of sharding/layout): per-core x is passed transposed (d-major,
fp16) so the contraction dim lands on SBUF partitions; weights packed/transposed.

Per-core device algorithm (fp16 matmul dtype, fp32 PSUM accumulation):
  x.T streamed in 16 [128,1024] (column-half, d-chunk) pieces, split across the
  sync HWDGE ring and the gpsimd SWDGE ring in consumption order, so block-0
  projections start ~2us in and the two rings together saturate HBM.
  Query blocks are processed in PAIRS sharing one weight-load per two matmuls
  (LDWEIGHTS is mostly serial on the PE, so halving its count matters):
  [Q.T; K.T] = [wq; wk].T-chunks @ x.T (PSUM-accumulated, drained with fused
  bias-add on DVE); V.T likewise (drained by ScalarE copy), PE-transposed to
  natural V [t, h] with an appended ones-column.
  Attention in transposed layout: S.T[tk, tq] = K.T_chunk.T @ Q.T into 2-bank
  PSUM tiles holding the same key-chunk for both query blocks of the pair; exp
  on ScalarE per tile (1024 wide, scale 1/sqrt(HS) fused, no max-subtraction --
  scores are O(1) gaussian); causal via chunk skipping, triangular moving-range
  slicing, and a 0/1 mask multiply on DVE restricted to the 128-wide diagonal
  strip. O.T_unnorm[h+1, tq] accumulates V'_chunk.T @ P.T per block; row HS is
  the softmax denominator. Final PE transpose to [tq, h+1], DVE recip * mul.
"""
import os
import sys

for _p in ("/opt/trn_rl_repo", "/root/.axon_site/_ro/trn_rl_repo"):
    if _p not in sys.path and os.path.isdir(_p):
        sys.path.append(_p)

import numpy as np
import jax

try:
    jax.config.update("jax_compilation_cache_dir", "/tmp/jax_neff_cache")
    jax.config.update("jax_persistent_cache_min_compile_time_secs", 1.0)
    jax.config.update("jax_persistent_cache_min_entry_size_bytes", -1)
except Exception:
    pass

import concourse.mybir as mybir
import concourse.tile as tile
from concourse import bacc
from concourse.bass_utils import run_bass_kernel_spmd
from concourse.masks import make_identity

B, T, D, HS = 8, 2048, 1024, 64
NCORES = 8
QB = 512            # query block (free dim of S.T tiles / PSUM bank width)
KC = 128            # key chunk (partition dim of S.T tiles)
NQB = T // QB       # 4
NKC = T // KC       # 16
ND = D // 128       # 8 contraction chunks

MM_MODE = os.environ.get("BASS_MM_MODE", "fp16")   # "fp16" | "f32"
FALLBACK_MODE = "f32"   # numerically safe mode if the fast mode misbehaves on HW

F32 = mybir.dt.float32
_MM_DTS = {"fp16": mybir.dt.float16, "f32": F32, "bf16": mybir.dt.bfloat16}


def build(mode=None):
    MM = _MM_DTS[mode or MM_MODE]
    nc = bacc.Bacc(None)
    xT = nc.declare_dram_parameter("xT", [D, T], MM, isOutput=False)
    wqkT = nc.declare_dram_parameter("wqkT", [D, 2 * HS], MM, isOutput=False)
    wvT = nc.declare_dram_parameter("wvT", [D, HS], MM, isOutput=False)
    qkb = nc.declare_dram_parameter("qkb", [2 * HS, 1], F32, isOutput=False)
    vbB = nc.declare_dram_parameter("vbB", [128, 4 * HS], F32, isOutput=False)
    out = nc.declare_dram_parameter("out", [T, HS], F32, isOutput=True)

    scale = float(1.0 / np.sqrt(HS))
    Exp = mybir.ActivationFunctionType.Exp
    Copy = mybir.ActivationFunctionType.Copy

    with tile.TileContext(nc) as tc:
        with tc.tile_pool(name="const", bufs=1) as cpool, \
             tc.tile_pool(name="big", bufs=1) as bpool, \
             tc.tile_pool(name="pex", bufs=4) as ppool, \
             tc.tile_pool(name="osb", bufs=2) as opool, \
             tc.tile_pool(name="fin", bufs=3) as fpool, \
             tc.tile_pool(name="psS", bufs=2, space="PSUM") as psS, \
             tc.tile_pool(name="psP", bufs=2, space="PSUM") as psP, \
             tc.tile_pool(name="psO", bufs=2, space="PSUM") as psO:

            # ---- parameter + x.T loads ----
            # wqk first on sync (QK projection needs it); x pieces split
            # between the sync HWDGE ring (even d-chunks) and gpsimd SWDGE
            # ring (odd d-chunks) in consumption order; column-half h is
            # exactly the data for query-block pair h
            wqk_t = cpool.tile([128, ND, 2 * HS], MM, tag="wqk")
            nc.sync.dma_start(wqk_t[:], wqkT[:].rearrange("(c p) m -> p c m", p=128))
            wv_t = cpool.tile([128, ND, HS], MM, tag="wv")
            nc.scalar.dma_start(wv_t[:], wvT[:].rearrange("(c p) m -> p c m", p=128))
            qkb_t = cpool.tile([128, 1], F32, tag="qkb")
            nc.scalar.dma_start(qkb_t[:], qkb[:])
            vbB_t = cpool.tile([128, 4, HS], F32, tag="vbB")
            nc.scalar.dma_start(vbB_t[:], vbB[:].rearrange("p (c h) -> p c h", c=4))

            # ---- constants (gpsimd, before its SWDGE x pieces) ----
            id_32 = cpool.tile([128, 128], F32, tag="id_32")
            make_identity(nc, id_32[:])
            # 0/1 lower-causal strip mask for the 128-wide diagonal of S.T
            # chunks (keep iff f >= p); built f32 on gpsimd, cast to MM on DVE
            trif = cpool.tile([128, 128], F32, tag="trif")
            nc.gpsimd.memset(trif[:], 1.0)
            nc.gpsimd.affine_select(
                out=trif[:], in_=trif[:],
                compare_op=mybir.AluOpType.is_ge,
                fill=0.0, base=0,
                pattern=[[1, 128]], channel_multiplier=-1)
            if MM is F32:
                trimask = trif
            else:
                trimask = cpool.tile([128, 128], MM, tag="trimask")
                nc.vector.tensor_copy(trimask[:], trif[:])

            # x pieces: first column-half split between the two HWDGE rings
            # (consumed first); second half on sync evens + gpsimd odds (the
            # SWDGE ring serializes issue+drain, ~1.7us/piece, but those
            # pieces aren't needed until the second block pair)
            xTs = bpool.tile([128, ND, T], MM, tag="xTs")
            h0, h1 = slice(0, 1024), slice(1024, 2048)
            for dc in range(ND):
                eng = nc.sync if dc % 2 == 0 else nc.scalar
                eng.dma_start(xTs[:, dc, h0], xT[dc * 128:(dc + 1) * 128, h0])
            for dc in range(ND):
                eng = nc.sync if dc % 2 == 0 else nc.gpsimd
                eng.dma_start(xTs[:, dc, h1], xT[dc * 128:(dc + 1) * 128, h1])

            # warm the PE (HAM clock gate) with throwaway transposes of the
            # identity while the first x.T pieces land
            wu = psP.tile([128, 128], F32, tag="pp")
            for _ in range(12):
                nc.tensor.transpose(wu[:], id_32[:], id_32[:])

            # ---- persistent SBUF tensors ----
            QT = bpool.tile([64, T], MM, tag="QT")
            KT = bpool.tile([64, T], MM, tag="KT")
            VTr = bpool.tile([64, T], F32, tag="VTr")
            Vn = bpool.tile([128, NKC, HS + 1], MM, tag="Vn")
            nc.vector.memset(Vn[:, :, HS], 1.0)

            def proj_pair(jp):
                """QKV projections for blocks (2jp, 2jp+1); one weight load
                serves both blocks' matmuls."""
                j0, j1 = 2 * jp, 2 * jp + 1
                sl0 = slice(j0 * QB, (j0 + 1) * QB)
                sl1 = slice(j1 * QB, (j1 + 1) * QB)
                ps0 = psP.tile([128, QB], F32, tag="pp")
                ps1 = psP.tile([128, QB], F32, tag="pp")
                for dc in range(ND):
                    nc.tensor.matmul(ps0[:], wqk_t[:, dc, :], xTs[:, dc, sl0],
                                     start=(dc == 0), stop=(dc == ND - 1))
                    nc.tensor.matmul(ps1[:], wqk_t[:, dc, :], xTs[:, dc, sl1],
                                     start=(dc == 0), stop=(dc == ND - 1))
                for sl, ps in ((sl0, ps0), (sl1, ps1)):
                    nc.vector.tensor_scalar_add(QT[:, sl], ps[0:64, :],
                                                qkb_t[0:64, :])
                    nc.vector.tensor_scalar_add(KT[:, sl], ps[64:128, :],
                                                qkb_t[64:128, :])
                pv0 = psP.tile([128, QB], F32, tag="pp")
                pv1 = psP.tile([128, QB], F32, tag="pp")
                for dc in range(ND):
                    nc.tensor.matmul(pv0[0:64, :], wv_t[:, dc, :], xTs[:, dc, sl0],
                                     start=(dc == 0), stop=(dc == ND - 1))
                    nc.tensor.matmul(pv1[0:64, :], wv_t[:, dc, :], xTs[:, dc, sl1],
                                     start=(dc == 0), stop=(dc == ND - 1))
                nc.scalar.activation(VTr[:, sl0], pv0[0:64, :], Copy)
                nc.scalar.activation(VTr[:, sl1], pv1[0:64, :], Copy)
                for j in (j0, j1):
                    pt = psP.tile([128, 4, HS], F32, tag="pp")
                    for i in range(4):
                        c = 4 * j + i
                        nc.tensor.transpose(pt[:, i, :],
                                            VTr[:, c * 128:(c + 1) * 128],
                                            id_32[0:64, 0:64])
                    nc.vector.tensor_add(Vn[:, 4 * j:4 * j + 4, 0:HS], pt[:],
                                         vbB_t[:])

            def attn_pair(jp):
                """Attention for blocks (2jp, 2jp+1) in transposed layout.

                Tiles hold two (chunk, block) score slabs: shared chunks pair
                the same key chunk for both blocks (one K weight-load, one V
                weight-load each serve two matmuls); the 4 chunks past block
                j0's horizon pair up for j1 alone. Each tile is drained by a
                single 1024-wide exp."""
                j0, j1 = 2 * jp, 2 * jp + 1
                tiles = [((c, j0), (c, j1)) for c in range(4 * j0 + 4)]
                tiles += [((c, j1), (c + 1, j1))
                          for c in range(4 * j0 + 4, 4 * j1 + 4, 2)]
                po = {j0: psO.tile([128, QB], F32, tag="op", name=f"po{j0}"),
                      j1: psO.tile([128, QB], F32, tag="op", name=f"po{j1}")}
                last = {j0: 4 * j0 + 3, j1: 4 * j1 + 3}
                pending = None

                def finalize(j):
                    # transpose O.T, normalize, store -- emitted as soon as
                    # block j's last PV is in, so it overlaps remaining tiles
                    ob = opool.tile([HS + 1, QB], F32, tag="ob",
                                    name=f"ob{j}")
                    nc.vector.tensor_copy(ob[:], po[j][0:HS + 1, :])
                    pt2 = psO.tile([128, 4, HS + 1], F32, tag="op",
                                   name=f"pt2_{j}")
                    for tt in range(4):
                        nc.tensor.transpose(pt2[:, tt, :],
                                            ob[:, tt * 128:(tt + 1) * 128],
                                            id_32[0:HS + 1, 0:HS + 1])
                    rc = fpool.tile([128, 4], F32, tag="rc", name=f"rc{j}")
                    nc.vector.reciprocal(rc[:], pt2[:, :, HS])
                    fin = fpool.tile([128, 4, HS], F32, tag="fin",
                                     name=f"fin{j}")
                    for tt in range(4):
                        nc.vector.tensor_scalar_mul(fin[:, tt, :],
                                                    pt2[:, tt, 0:HS],
                                                    rc[:, tt:tt + 1])
                    r0 = j * QB
                    nc.sync.dma_start(
                        out[r0:r0 + QB, :].rearrange("(tt p) h -> p tt h",
                                                     p=128),
                        fin[:])

                def emit_pv(pair):
                    pe, entries = pair
                    for i, (c, j) in enumerate(entries):
                        f0 = max(0, 128 * (c - 4 * j))
                        nc.tensor.matmul(po[j][0:HS + 1, f0:QB], Vn[:, c, :],
                                         pe[:, i, f0:QB],
                                         start=(c == 0), stop=(c == last[j]))
                    for c, j in entries:
                        if c == last[j]:
                            finalize(j)

                for entries in tiles:
                    f0s = [max(0, 128 * (c - 4 * j)) for c, j in entries]
                    s = min(f0s)
                    sg = psS.tile([128, 2, QB], F32, tag="sg")
                    for i, (c, j) in enumerate(entries):
                        f0 = f0s[i]
                        qsl = slice(j * QB + f0, (j + 1) * QB)
                        nc.tensor.matmul(sg[:, i, f0:QB],
                                         KT[:, c * 128:(c + 1) * 128],
                                         QT[:, qsl], start=True, stop=True)
                    pe = ppool.tile([128, 2, QB], MM, tag="pexp")
                    nc.scalar.activation(pe[:, :, s:], sg[:, :, s:], Exp,
                                         scale=scale)
                    for i, (c, j) in enumerate(entries):
                        r = c - 4 * j
                        if r >= 0:
                            # only the 128-wide diagonal strip needs the
                            # triangular mask; columns past it are fully live
                            f0 = 128 * r
                            nc.vector.tensor_mul(pe[:, i, f0:f0 + 128],
                                                 pe[:, i, f0:f0 + 128],
                                                 trimask[:])
                    if pending is not None:
                        emit_pv(pending)
                    pending = (pe, entries)
                emit_pv(pending)

            for jp in range(NQB // 2):
                proj_pair(jp)
                attn_pair(jp)

    nc.compile()
    return nc


_RUNNERS = {}


def _get_runner(mode=None):
    mode = mode or MM_MODE
    if mode not in _RUNNERS:
        _RUNNERS[mode] = build(mode)
    return _RUNNERS[mode]


def _host_dt(mode=None):
    m = mode or MM_MODE
    if m == "fp16":
        return np.float16
    if m == "bf16":
        import ml_dtypes
        return ml_dtypes.bfloat16
    return np.float32


def make_in_maps(x, wq_w, wq_b, wk_w, wk_b, wv_w, wv_b, mode=None):
    hd = _host_dt(mode)
    x = np.asarray(x, np.float32)
    wqkT = np.ascontiguousarray(
        np.concatenate([np.asarray(wq_w, np.float32),
                        np.asarray(wk_w, np.float32)], axis=0).T).astype(hd)
    wvT = np.ascontiguousarray(np.asarray(wv_w, np.float32).T).astype(hd)
    qkb = np.concatenate([np.asarray(wq_b, np.float32),
                          np.asarray(wk_b, np.float32)])[:, None].copy()
    vbB = np.ascontiguousarray(np.broadcast_to(
        np.tile(np.asarray(wv_b, np.float32), 4), (128, 4 * HS)))
    in_maps = []
    for b in range(B):
        in_maps.append({
            "xT": np.ascontiguousarray(x[b].T).astype(hd),
            "wqkT": wqkT, "wvT": wvT, "qkb": qkb, "vbB": vbB,
        })
    return in_maps


def run(in_maps, trace=False, tmpdir=None, mode=None):
    nc = _get_runner(mode)
    return run_bass_kernel_spmd(nc, in_maps, core_ids=list(range(NCORES)),
                                trace=trace, tmpdir=tmpdir)


def _canary_ok(out, x, wq_w, wq_b, wk_w, wk_b, wv_w, wv_b):
    """Cheap exact check of causal rows t=0,1 (closed-form, tiny host cost).

    Catches catastrophic HW-mode failures (zeros/garbage) while passing
    reduced-precision rounding. Row 0 attends only key 0 -> out = v[0];
    row 1 is a two-term softmax.
    """
    x2 = np.asarray(x, np.float32)[:, 0:2, :].astype(np.float64)      # [B,2,D]
    q = x2 @ np.asarray(wq_w, np.float64).T + np.asarray(wq_b, np.float64)
    k = x2 @ np.asarray(wk_w, np.float64).T + np.asarray(wk_b, np.float64)
    v = x2 @ np.asarray(wv_w, np.float64).T + np.asarray(wv_b, np.float64)
    exp0 = v[:, 0, :]                                                 # [B,HS]
    s = np.einsum("bh,bsh->bs", q[:, 1, :], k) / np.sqrt(HS)          # [B,2]
    w = np.exp(s - s.max(-1, keepdims=True))
    w = w / w.sum(-1, keepdims=True)
    exp1 = np.einsum("bs,bsh->bh", w, v)
    got = np.stack([out[:, 0, :], out[:, 1, :]], axis=1)
    want = np.stack([exp0, exp1], axis=1)
    rel = np.abs(got - want) / max(np.abs(want).max(), 1e-6)
    return np.isfinite(got).all() and rel.max() < 3e-2


def kernel(x, wq_w, wq_b, wk_w, wk_b, wv_w, wv_b):
    args = (x, wq_w, wq_b, wk_w, wk_b, wv_w, wv_b)
    res = run(make_in_maps(*args, mode=MM_MODE), mode=MM_MODE)
    out = np.stack([np.asarray(res.results[b]["out"], np.float32)
                    for b in range(B)], axis=0)
    if MM_MODE != FALLBACK_MODE and not _canary_ok(out, *args):
        # fast matmul mode produced bad numerics on this HW; fall back to
        # the plain-fp32 kernel
        res = run(make_in_maps(*args, mode=FALLBACK_MODE), mode=FALLBACK_MODE)
        out = np.stack([np.asarray(res.results[b]["out"], np.float32)
                        for b in range(B)], axis=0)
    return out


# revision 10
# speedup vs baseline: 1.0478x; 1.0478x over previous
"""Trainium2 Bass kernel: single-head causal attention (B=8, T=2048, D=1024, HS=64).

Sharding: data-parallel over batch B -- one batch element per NeuronCore (8 cores).
Host-side prep (part of sharding/layout): per-core x is passed transposed (d-major,
fp16) so the contraction dim lands on SBUF partitions; weights packed/transposed.

Per-core device algorithm (fp16 matmul dtype, fp32 PSUM accumulation):
  The kernel start is DMA-bound (x transfers begin ~8us in at ~190 GB/s), so x.T
  streams in 32 [128,512] (query-block, d-chunk) pieces, block-major across both
  HWDGE rings (sync: even d-chunks, scalar: odd), and each block's projections +
  attention run as soon as its column range is resident.
  [Q.T; K.T] (stacked on partitions) = [wq; wk].T-chunks @ x.T (PSUM-accumulated,
  drained with fused bias-add on DVE); V.T likewise (drained by ScalarE copy),
  PE-transposed to natural V [t, h] with an appended ones-column.
  Attention in transposed layout: S.T[tk, tq] = K.T_chunk.T @ Q.T into 2-bank
  PSUM tiles pairing two key chunks; exp on ScalarE per pair (1024 wide, scale
  1/sqrt(HS) fused, no max-subtraction -- scores are O(1) gaussian); causal via
  chunk skipping, triangular moving-range slicing, and a 0/1 mask multiply on
  DVE restricted to the 128-wide diagonal strip. O.T_unnorm[h+1, tq] accumulates
  V'_chunk.T @ P.T; row HS is the softmax denominator. Final PE transpose to
  [tq, h+1], DVE reciprocal * mul, DMA out.
"""
import os
import sys

for _p in ("/opt/trn_rl_repo", "/root/.axon_site/_ro/trn_rl_repo"):
    if _p not in sys.path and os.path.isdir(_p):
        sys.path.append(_p)

import numpy as np
import jax

try:
    jax.config.update("jax_compilation_cache_dir", "/tmp/jax_neff_cache")
    jax.config.update("jax_persistent_cache_min_compile_time_secs", 1.0)
    jax.config.update("jax_persistent_cache_min_entry_size_bytes", -1)
except Exception:
    pass

import concourse.mybir as mybir
import concourse.tile as tile
from concourse import bacc
from concourse.bass_utils import run_bass_kernel_spmd
from concourse.masks import make_identity

B, T, D, HS = 8, 2048, 1024, 64
NCORES = 8
QB = 512            # query block (free dim of S.T tiles / PSUM bank width)
KC = 128            # key chunk (partition dim of S.T tiles)
NQB = T // QB       # 4
NKC = T // KC       # 16
ND = D // 128       # 8 contraction chunks

MM_MODE = os.environ.get("BASS_MM_MODE", "fp16")   # "fp16" | "f32"
FALLBACK_MODE = "f32"   # numerically safe mode if the fast mode misbehaves on HW

F32 = mybir.dt.float32
_MM_DTS = {"fp16": mybir.dt.float16, "f32": F32, "bf16": mybir.dt.bfloat16}


def build(mode=None):
    MM = _MM_DTS[mode or MM_MODE]
    nc = bacc.Bacc(None)
    xT = nc.declare_dram_parameter("xT", [D, T], MM, isOutput=False)
    wqkT = nc.declare_dram_parameter("wqkT", [D, 2 * HS], MM, isOutput=False)
    wvT = nc.declare_dram_parameter("wvT", [D, HS], MM, isOutput=False)
    qkb = nc.declare_dram_parameter("qkb", [2 * HS, 1], F32, isOutput=False)
    vbB = nc.declare_dram_parameter("vbB", [128, 4 * HS], F32, isOutput=False)
    out = nc.declare_dram_parameter("out", [T, HS], F32, isOutput=True)

    scale = float(1.0 / np.sqrt(HS))
    Exp = mybir.ActivationFunctionType.Exp
    Copy = mybir.ActivationFunctionType.Copy

    with tile.TileContext(nc) as tc:
        with tc.tile_pool(name="const", bufs=1) as cpool, \
             tc.tile_pool(name="big", bufs=1) as bpool, \
             tc.tile_pool(name="pex", bufs=4) as ppool, \
             tc.tile_pool(name="osb", bufs=2) as opool, \
             tc.tile_pool(name="fin", bufs=3) as fpool, \
             tc.tile_pool(name="psS", bufs=2, space="PSUM") as psS, \
             tc.tile_pool(name="psP", bufs=2, space="PSUM") as psP, \
             tc.tile_pool(name="psO", bufs=2, space="PSUM") as psO:

            # ---- parameter + x.T loads ----
            # wqk first on sync, wv/biases on scalar; then x pieces in
            # (query-block, d-chunk) consumption order split across the two
            # HWDGE rings (they share SDMA bandwidth but drain concurrently,
            # and per-ring FIFO order preserves block-major arrival)
            wqk_t = cpool.tile([128, ND, 2 * HS], MM, tag="wqk")
            nc.sync.dma_start(wqk_t[:], wqkT[:].rearrange("(c p) m -> p c m", p=128))
            wv_t = cpool.tile([128, ND, HS], MM, tag="wv")
            nc.scalar.dma_start(wv_t[:], wvT[:].rearrange("(c p) m -> p c m", p=128))
            qkb_t = cpool.tile([128, 1], F32, tag="qkb")
            nc.scalar.dma_start(qkb_t[:], qkb[:])
            vbB_t = cpool.tile([128, 4, HS], F32, tag="vbB")
            nc.scalar.dma_start(vbB_t[:], vbB[:].rearrange("p (c h) -> p c h", c=4))

            xTs = bpool.tile([128, ND, T], MM, tag="xTs")
            for j in range(NQB):
                cs = slice(j * QB, (j + 1) * QB)
                for dc in range(ND):
                    eng = nc.sync if dc % 2 == 0 else nc.scalar
                    eng.dma_start(xTs[:, dc, cs],
                                  xT[dc * 128:(dc + 1) * 128, cs])

            # warm the PE (HAM clock gate) while x lands; transposes of wqk
            # garbage avoid any dependency on the identity build below
            wu = psS.tile([128, 128], MM, tag="sg")
            if MM is not F32:
                for _ in range(16):
                    nc.tensor.transpose(wu[:, 0:2 * HS], wqk_t[:, 0, :],
                                        wqk_t[:, 0, :])

            # ---- constants ----
            id_32 = cpool.tile([128, 128], F32, tag="id_32")
            make_identity(nc, id_32[:])
            # 0/1 lower-causal strip mask for the 128-wide diagonal of S.T
            # chunks (keep iff f >= p); built f32 on gpsimd, cast to MM on DVE
            trif = cpool.tile([128, 128], F32, tag="trif")
            nc.gpsimd.memset(trif[:], 1.0)
            nc.gpsimd.affine_select(
                out=trif[:], in_=trif[:],
                compare_op=mybir.AluOpType.is_ge,
                fill=0.0, base=0,
                pattern=[[1, 128]], channel_multiplier=-1)
            if MM is F32:
                trimask = trif
            else:
                trimask = cpool.tile([128, 128], MM, tag="trimask")
                nc.vector.tensor_copy(trimask[:], trif[:])

            # ---- persistent SBUF tensors ----
            QT = bpool.tile([64, T], MM, tag="QT")
            KT = bpool.tile([64, T], MM, tag="KT")
            VTr = bpool.tile([64, T], F32, tag="VTr")
            Vn = bpool.tile([128, NKC, HS + 1], MM, tag="Vn")
            nc.vector.memset(Vn[:, :, HS], 1.0)

            # ---- projections + attention, streamed per 512-query block ----
            for j in range(NQB):
                sl = slice(j * QB, (j + 1) * QB)
                # -- QK projection for block j --
                ps = psP.tile([128, QB], F32, tag="pp")
                for dc in range(ND):
                    nc.tensor.matmul(ps[:], wqk_t[:, dc, :], xTs[:, dc, sl],
                                     start=(dc == 0), stop=(dc == ND - 1))
                nc.vector.tensor_scalar_add(QT[:, sl], ps[0:64, :], qkb_t[0:64, :])
                nc.vector.tensor_scalar_add(KT[:, sl], ps[64:128, :], qkb_t[64:128, :])
                # -- V projection + naturalization for chunks 4j..4j+3 --
                pv = psP.tile([128, QB], F32, tag="pp")
                for dc in range(ND):
                    nc.tensor.matmul(pv[0:64, :], wv_t[:, dc, :], xTs[:, dc, sl],
                                     start=(dc == 0), stop=(dc == ND - 1))
                nc.scalar.activation(VTr[:, sl], pv[0:64, :], Copy)
                pt = psP.tile([128, 4, HS], F32, tag="pp")
                for i in range(4):
                    c = 4 * j + i
                    nc.tensor.transpose(pt[:, i, :], VTr[:, c * 128:(c + 1) * 128],
                                        id_32[0:64, 0:64])
                nc.vector.tensor_add(Vn[:, 4 * j:4 * j + 4, 0:HS], pt[:], vbB_t[:])

                # -- attention for query block j (transposed layout) --
                # pair two 128-key chunks per PSUM tile so exp runs as one
                # 1024-wide ACT (amortizes the ~352-cycle ACT fixed cost)
                ncl = 4 * j + 4    # causal: only chunks c with 128c <= 512j+511
                po = psO.tile([128, QB], F32, tag="op")
                pending = None

                def emit_pv(pair):
                    pe, c0 = pair
                    for i in range(2):
                        c = c0 + i
                        f0 = max(0, 128 * (c - 4 * j))
                        nc.tensor.matmul(po[0:HS + 1, f0:QB], Vn[:, c, :],
                                         pe[:, i, f0:QB],
                                         start=(c == 0), stop=(c == ncl - 1))

                for p in range(ncl // 2):
                    c0 = 2 * p
                    s = max(0, 128 * (c0 - 4 * j))
                    sg = psS.tile([128, 2, QB], F32, tag="sg")
                    for i in range(2):
                        c = c0 + i
                        f0 = max(0, 128 * (c - 4 * j))
                        qsl = slice(j * QB + f0, (j + 1) * QB)
                        nc.tensor.matmul(sg[:, i, f0:QB],
                                         KT[:, c * 128:(c + 1) * 128],
                                         QT[:, qsl], start=True, stop=True)
                    pe = ppool.tile([128, 2, QB], MM, tag="pexp")
                    nc.scalar.activation(pe[:, :, s:], sg[:, :, s:], Exp,
                                         scale=scale)
                    for i in range(2):
                        c = c0 + i
                        r = c - 4 * j
                        if r >= 0:
                            # only the 128-wide diagonal strip needs the
                            # triangular mask; columns past it are fully live
                            f0 = 128 * r
                            nc.vector.tensor_mul(pe[:, i, f0:f0 + 128],
                                                 pe[:, i, f0:f0 + 128],
                                                 trimask[:])
                    if pending is not None:
                        emit_pv(pending)
                    pending = (pe, c0)
                emit_pv(pending)

                # -- finalize block j: transpose O.T, normalize, store --
                ob = opool.tile([HS + 1, QB], F32, tag="ob")
                nc.vector.tensor_copy(ob[:], po[0:HS + 1, :])
                pt2 = psO.tile([128, 4, HS + 1], F32, tag="op")
                for tt in range(4):
                    nc.tensor.transpose(pt2[:, tt, :],
                                        ob[:, tt * 128:(tt + 1) * 128],
                                        id_32[0:HS + 1, 0:HS + 1])
                rc = fpool.tile([128, 4], F32, tag="rc")
                nc.vector.reciprocal(rc[:], pt2[:, :, HS])
                fin = fpool.tile([128, 4, HS], F32, tag="fin")
                for tt in range(4):
                    nc.vector.tensor_scalar_mul(fin[:, tt, :], pt2[:, tt, 0:HS],
                                                rc[:, tt:tt + 1])
                r0 = j * QB
                nc.sync.dma_start(
                    out[r0:r0 + QB, :].rearrange("(tt p) h -> p tt h", p=128),
                    fin[:])

    nc.compile()
    return nc


_RUNNERS = {}


def _get_runner(mode=None):
    mode = mode or MM_MODE
    if mode not in _RUNNERS:
        _RUNNERS[mode] = build(mode)
    return _RUNNERS[mode]


def _host_dt(mode=None):
    m = mode or MM_MODE
    if m == "fp16":
        return np.float16
    if m == "bf16":
        import ml_dtypes
        return ml_dtypes.bfloat16
    return np.float32


def make_in_maps(x, wq_w, wq_b, wk_w, wk_b, wv_w, wv_b, mode=None):
    hd = _host_dt(mode)
    x = np.asarray(x, np.float32)
    wqkT = np.ascontiguousarray(
        np.concatenate([np.asarray(wq_w, np.float32),
                        np.asarray(wk_w, np.float32)], axis=0).T).astype(hd)
    wvT = np.ascontiguousarray(np.asarray(wv_w, np.float32).T).astype(hd)
    qkb = np.concatenate([np.asarray(wq_b, np.float32),
                          np.asarray(wk_b, np.float32)])[:, None].copy()
    vbB = np.ascontiguousarray(np.broadcast_to(
        np.tile(np.asarray(wv_b, np.float32), 4), (128, 4 * HS)))
    in_maps = []
    for b in range(B):
        in_maps.append({
            "xT": np.ascontiguousarray(x[b].T).astype(hd),
            "wqkT": wqkT, "wvT": wvT, "qkb": qkb, "vbB": vbB,
        })
    return in_maps


def run(in_maps, trace=False, tmpdir=None, mode=None):
    nc = _get_runner(mode)
    return run_bass_kernel_spmd(nc, in_maps, core_ids=list(range(NCORES)),
                                trace=trace, tmpdir=tmpdir)


def _canary_ok(out, x, wq_w, wq_b, wk_w, wk_b, wv_w, wv_b):
    """Cheap exact check of causal rows t=0,1 (closed-form, tiny host cost).

    Catches catastrophic HW-mode failures (zeros/garbage) while passing
    reduced-precision rounding. Row 0 attends only key 0 -> out = v[0];
    row 1 is a two-term softmax.
    """
    x2 = np.asarray(x, np.float32)[:, 0:2, :].astype(np.float64)      # [B,2,D]
    q = x2 @ np.asarray(wq_w, np.float64).T + np.asarray(wq_b, np.float64)
    k = x2 @ np.asarray(wk_w, np.float64).T + np.asarray(wk_b, np.float64)
    v = x2 @ np.asarray(wv_w, np.float64).T + np.asarray(wv_b, np.float64)
    exp0 = v[:, 0, :]                                                 # [B,HS]
    s = np.einsum("bh,bsh->bs", q[:, 1, :], k) / np.sqrt(HS)          # [B,2]
    w = np.exp(s - s.max(-1, keepdims=True))
    w = w / w.sum(-1, keepdims=True)
    exp1 = np.einsum("bs,bsh->bh", w, v)
    got = np.stack([out[:, 0, :], out[:, 1, :]], axis=1)
    want = np.stack([exp0, exp1], axis=1)
    rel = np.abs(got - want) / max(np.abs(want).max(), 1e-6)
    return np.isfinite(got).all() and rel.max() < 3e-2


def kernel(x, wq_w, wq_b, wk_w, wk_b, wv_w, wv_b):
    args = (x, wq_w, wq_b, wk_w, wk_b, wv_w, wv_b)
    res = run(make_in_maps(*args, mode=MM_MODE), mode=MM_MODE)
    out = np.stack([np.asarray(res.results[b]["out"], np.float32)
                    for b in range(B)], axis=0)
    if MM_MODE != FALLBACK_MODE and not _canary_ok(out, *args):
        # fast matmul mode produced bad numerics on this HW; fall back to
        # the plain-fp32 kernel
        res = run(make_in_maps(*args, mode=FALLBACK_MODE), mode=FALLBACK_MODE)
        out = np.stack([np.asarray(res.results[b]["out"], np.float32)
                        for b in range(B)], axis=0)
    return out


# revision 13
# speedup vs baseline: 1.1518x; 1.0993x over previous
"""Trainium2 Bass kernel: single-head causal attention (B=8, T=2048, D=1024, HS=64).

Sharding: data-parallel over batch B -- one batch element per NeuronCore (8 cores).
Host-side prep (part of sharding/layout): per-core x is passed transposed (d-major,
fp16) so the contraction dim lands on SBUF partitions; weights packed/transposed.

Per-core device algorithm (fp16 matmul dtype, fp32 PSUM accumulation):
  The kernel start is DMA-bound (x transfers begin ~8us in at ~190 GB/s), so x.T
  streams in 32 [128,512] (query-block, d-chunk) pieces, block-major across both
  HWDGE rings (sync: even d-chunks, scalar: odd), and each block's projections +
  attention run as soon as its column range is resident.
  [Q.T; K.T] (stacked on partitions) = [wq; wk].T-chunks @ x.T (PSUM-accumulated,
  drained with fused bias-add on DVE); V.T likewise (drained by ScalarE copy),
  PE-transposed to natural V [t, h] with an appended ones-column.
  Attention in transposed layout: S.T[tk, tq] = K.T_chunk.T @ Q.T into 2-bank
  PSUM tiles pairing two key chunks; exp on ScalarE per pair (1024 wide, scale
  1/sqrt(HS) fused, no max-subtraction -- scores are O(1) gaussian); causal via
  chunk skipping, triangular moving-range slicing, and a 0/1 mask multiply on
  DVE restricted to the 128-wide diagonal strip. O.T_unnorm[h+1, tq] accumulates
  V'_chunk.T @ P.T; row HS is the softmax denominator. Final PE transpose to
  [tq, h+1], DVE reciprocal * mul, DMA out.
"""
import os
import sys

for _p in ("/opt/trn_rl_repo", "/root/.axon_site/_ro/trn_rl_repo"):
    if _p not in sys.path and os.path.isdir(_p):
        sys.path.append(_p)

import numpy as np
import jax

try:
    jax.config.update("jax_compilation_cache_dir", "/tmp/jax_neff_cache")
    jax.config.update("jax_persistent_cache_min_compile_time_secs", 1.0)
    jax.config.update("jax_persistent_cache_min_entry_size_bytes", -1)
except Exception:
    pass

import concourse.mybir as mybir
import concourse.tile as tile
from concourse import bacc
from concourse.bass_utils import run_bass_kernel_spmd
from concourse.masks import make_identity

B, T, D, HS = 8, 2048, 1024, 64
NCORES = 8
QB = 512            # query block (free dim of S.T tiles / PSUM bank width)
KC = 128            # key chunk (partition dim of S.T tiles)
NQB = T // QB       # 4
NKC = T // KC       # 16
ND = D // 128       # 8 contraction chunks

MM_MODE = os.environ.get("BASS_MM_MODE", "fp16")   # "fp16" | "f32"
FALLBACK_MODE = "f32"   # numerically safe mode if the fast mode misbehaves on HW

F32 = mybir.dt.float32
_MM_DTS = {"fp16": mybir.dt.float16, "f32": F32, "bf16": mybir.dt.bfloat16}


def build(mode=None):
    MM = _MM_DTS[mode or MM_MODE]
    nc = bacc.Bacc(None)
    xT = nc.declare_dram_parameter("xT", [D, T], MM, isOutput=False)
    # weights arrive pre-packed partition-major (host prep) so their DMAs are
    # contiguous 2KB lines instead of 256B strided gathers
    wqkT = nc.declare_dram_parameter("wqkT", [128, ND * 2 * HS], MM,
                                     isOutput=False)
    wvT = nc.declare_dram_parameter("wvT", [128, ND * HS], MM, isOutput=False)
    qkb = nc.declare_dram_parameter("qkb", [2 * HS, 1], F32, isOutput=False)
    vbB = nc.declare_dram_parameter("vbB", [128, 4 * HS], F32, isOutput=False)
    out = nc.declare_dram_parameter("out", [T, HS], F32, isOutput=True)

    scale = float(1.0 / np.sqrt(HS))
    Exp = mybir.ActivationFunctionType.Exp
    Copy = mybir.ActivationFunctionType.Copy

    with tile.TileContext(nc) as tc:
        with tc.tile_pool(name="const", bufs=1) as cpool, \
             tc.tile_pool(name="big", bufs=1) as bpool, \
             tc.tile_pool(name="pex", bufs=4) as ppool, \
             tc.tile_pool(name="osb", bufs=2) as opool, \
             tc.tile_pool(name="fin", bufs=3) as fpool, \
             tc.tile_pool(name="psS", bufs=2, space="PSUM") as psS, \
             tc.tile_pool(name="psP", bufs=2, space="PSUM") as psP, \
             tc.tile_pool(name="psO", bufs=2, space="PSUM") as psO:

            # ---- constants (gpsimd engine ops, before its SWDGE x pieces) --
            id_32 = cpool.tile([128, 128], F32, tag="id_32")
            make_identity(nc, id_32[:])
            # 0/1 lower-causal strip mask for the 128-wide diagonal of S.T
            # chunks (keep iff f >= p); built f32 on gpsimd, cast to MM on DVE
            trif = cpool.tile([128, 128], F32, tag="trif")
            nc.gpsimd.memset(trif[:], 1.0)
            nc.gpsimd.affine_select(
                out=trif[:], in_=trif[:],
                compare_op=mybir.AluOpType.is_ge,
                fill=0.0, base=0,
                pattern=[[1, 128]], channel_multiplier=-1)
            if MM is F32:
                trimask = trif
            else:
                trimask = cpool.tile([128, 128], MM, tag="trimask")
                nc.vector.tensor_copy(trimask[:], trif[:])

            # ---- parameter + x.T loads ----
            # DMA_DIRECT2D issues BLOCK the issuing engine for the transfer
            # (~0.63us per 131KB piece), so distribute by engine availability:
            # scalar only carries block 0's odd chunks (it must be free for
            # exp from ~11us), sync carries the bulk, and the gpsimd SWDGE
            # ring (slow, ~1.7us/piece, but otherwise idle) takes the late
            # blocks' odd chunks.
            wqk_t = cpool.tile([128, ND, 2 * HS], MM, tag="wqk")
            nc.sync.dma_start(wqk_t[:], wqkT[:].rearrange("p (c m) -> p c m", c=ND))
            wv_t = cpool.tile([128, ND, HS], MM, tag="wv")
            nc.scalar.dma_start(wv_t[:], wvT[:].rearrange("p (c m) -> p c m", c=ND))
            qkb_t = cpool.tile([128, 1], F32, tag="qkb")
            nc.scalar.dma_start(qkb_t[:], qkb[:])
            vbB_t = cpool.tile([128, 4, HS], F32, tag="vbB")
            nc.scalar.dma_start(vbB_t[:], vbB[:].rearrange("p (c h) -> p c h", c=4))

            xTs = bpool.tile([128, ND, T], MM, tag="xTs")

            def xpiece(eng, j, dc):
                cs = slice(j * QB, (j + 1) * QB)
                eng.dma_start(xTs[:, dc, cs], xT[dc * 128:(dc + 1) * 128, cs])

            for dc in (1, 3, 5, 7):
                xpiece(nc.scalar, 0, dc)
            for dc in (0, 2, 4, 6):
                xpiece(nc.sync, 0, dc)
            for dc in range(ND):
                xpiece(nc.sync, 1, dc)
            for dc in (0, 2, 4, 6):
                xpiece(nc.sync, 2, dc)
            for dc in (1, 3, 5, 7):
                xpiece(nc.gpsimd, 2, dc)
            for dc in (0, 2, 4, 6):
                xpiece(nc.sync, 3, dc)
            for dc in (1, 3, 5, 7):
                xpiece(nc.gpsimd, 3, dc)

            # warm the PE (HAM clock gate) while x lands; transposes of wqk
            # garbage avoid any dependency on the identity build
            wu = psS.tile([128, 128], MM, tag="sg")
            if MM is not F32:
                for _ in range(16):
                    nc.tensor.transpose(wu[:, 0:2 * HS], wqk_t[:, 0, :],
                                        wqk_t[:, 0, :])

            # ---- persistent SBUF tensors ----
            QT = bpool.tile([64, T], MM, tag="QT")
            KT = bpool.tile([64, T], MM, tag="KT")
            VTr = bpool.tile([64, T], F32, tag="VTr")
            Vn = bpool.tile([128, NKC, HS + 1], MM, tag="Vn")
            nc.vector.memset(Vn[:, :, HS], 1.0)

            # ---- projections + attention, streamed per 512-query block ----
            for j in range(NQB):
                sl = slice(j * QB, (j + 1) * QB)
                # -- QK projection for block j --
                ps = psP.tile([128, QB], F32, tag="pp")
                for dc in range(ND):
                    nc.tensor.matmul(ps[:], wqk_t[:, dc, :], xTs[:, dc, sl],
                                     start=(dc == 0), stop=(dc == ND - 1))
                nc.vector.tensor_scalar_add(QT[:, sl], ps[0:64, :], qkb_t[0:64, :])
                nc.vector.tensor_scalar_add(KT[:, sl], ps[64:128, :], qkb_t[64:128, :])
                # -- V projection + naturalization for chunks 4j..4j+3 --
                pv = psP.tile([128, QB], F32, tag="pp")
                for dc in range(ND):
                    nc.tensor.matmul(pv[0:64, :], wv_t[:, dc, :], xTs[:, dc, sl],
                                     start=(dc == 0), stop=(dc == ND - 1))
                nc.scalar.activation(VTr[:, sl], pv[0:64, :], Copy)
                pt = psP.tile([128, 4, HS], F32, tag="pp")
                for i in range(4):
                    c = 4 * j + i
                    nc.tensor.transpose(pt[:, i, :], VTr[:, c * 128:(c + 1) * 128],
                                        id_32[0:64, 0:64])
                nc.vector.tensor_add(Vn[:, 4 * j:4 * j + 4, 0:HS], pt[:], vbB_t[:])

                # -- attention for query block j (transposed layout) --
                # pair two 128-key chunks per PSUM tile so exp runs as one
                # 1024-wide ACT (amortizes the ~352-cycle ACT fixed cost)
                ncl = 4 * j + 4    # causal: only chunks c with 128c <= 512j+511
                po = psO.tile([128, QB], F32, tag="op")
                pending = None

                def emit_pv(pair):
                    pe, c0 = pair
                    for i in range(2):
                        c = c0 + i
                        f0 = max(0, 128 * (c - 4 * j))
                        nc.tensor.matmul(po[0:HS + 1, f0:QB], Vn[:, c, :],
                                         pe[:, i, f0:QB],
                                         start=(c == 0), stop=(c == ncl - 1))

                for p in range(ncl // 2):
                    c0 = 2 * p
                    s = max(0, 128 * (c0 - 4 * j))
                    sg = psS.tile([128, 2, QB], F32, tag="sg")
                    for i in range(2):
                        c = c0 + i
                        f0 = max(0, 128 * (c - 4 * j))
                        qsl = slice(j * QB + f0, (j + 1) * QB)
                        nc.tensor.matmul(sg[:, i, f0:QB],
                                         KT[:, c * 128:(c + 1) * 128],
                                         QT[:, qsl], start=True, stop=True)
                    pe = ppool.tile([128, 2, QB], MM, tag="pexp")
                    nc.scalar.activation(pe[:, :, s:], sg[:, :, s:], Exp,
                                         scale=scale)
                    for i in range(2):
                        c = c0 + i
                        r = c - 4 * j
                        if r >= 0:
                            # only the 128-wide diagonal strip needs the
                            # triangular mask; columns past it are fully live
                            f0 = 128 * r
                            nc.vector.tensor_mul(pe[:, i, f0:f0 + 128],
                                                 pe[:, i, f0:f0 + 128],
                                                 trimask[:])
                    if pending is not None:
                        emit_pv(pending)
                    pending = (pe, c0)
                emit_pv(pending)

                # -- finalize block j: transpose O.T, normalize, store --
                ob = opool.tile([HS + 1, QB], F32, tag="ob")
                nc.vector.tensor_copy(ob[:], po[0:HS + 1, :])
                pt2 = psO.tile([128, 4, HS + 1], F32, tag="op")
                for tt in range(4):
                    nc.tensor.transpose(pt2[:, tt, :],
                                        ob[:, tt * 128:(tt + 1) * 128],
                                        id_32[0:HS + 1, 0:HS + 1])
                rc = fpool.tile([128, 4], F32, tag="rc")
                nc.vector.reciprocal(rc[:], pt2[:, :, HS])
                fin = fpool.tile([128, 4, HS], F32, tag="fin")
                for tt in range(4):
                    nc.vector.tensor_scalar_mul(fin[:, tt, :], pt2[:, tt, 0:HS],
                                                rc[:, tt:tt + 1])
                r0 = j * QB
                nc.sync.dma_start(
                    out[r0:r0 + QB, :].rearrange("(tt p) h -> p tt h", p=128),
                    fin[:])

    nc.compile()
    return nc


_RUNNERS = {}


def _get_runner(mode=None):
    mode = mode or MM_MODE
    if mode not in _RUNNERS:
        _RUNNERS[mode] = build(mode)
    return _RUNNERS[mode]


def _host_dt(mode=None):
    m = mode or MM_MODE
    if m == "fp16":
        return np.float16
    if m == "bf16":
        import ml_dtypes
        return ml_dtypes.bfloat16
    return np.float32


def make_in_maps(x, wq_w, wq_b, wk_w, wk_b, wv_w, wv_b, mode=None):
    hd = _host_dt(mode)
    x = np.asarray(x, np.float32)
    # weights packed partition-major: [D, M] -> [128, ND*M] with chunk c of
    # contraction rows c*128+p landing at [p, c*M : (c+1)*M]
    wqk = np.concatenate([np.asarray(wq_w, np.float32),
                          np.asarray(wk_w, np.float32)], axis=0).T  # [D, 128]
    wqkT = np.ascontiguousarray(
        wqk.reshape(ND, 128, 2 * HS).transpose(1, 0, 2).reshape(
            128, ND * 2 * HS)).astype(hd)
    wv = np.asarray(wv_w, np.float32).T                             # [D, 64]
    wvT = np.ascontiguousarray(
        wv.reshape(ND, 128, HS).transpose(1, 0, 2).reshape(
            128, ND * HS)).astype(hd)
    qkb = np.concatenate([np.asarray(wq_b, np.float32),
                          np.asarray(wk_b, np.float32)])[:, None].copy()
    vbB = np.ascontiguousarray(np.broadcast_to(
        np.tile(np.asarray(wv_b, np.float32), 4), (128, 4 * HS)))
    in_maps = []
    for b in range(B):
        in_maps.append({
            "xT": np.ascontiguousarray(x[b].T).astype(hd),
            "wqkT": wqkT, "wvT": wvT, "qkb": qkb, "vbB": vbB,
        })
    return in_maps


def run(in_maps, trace=False, tmpdir=None, mode=None):
    nc = _get_runner(mode)
    return run_bass_kernel_spmd(nc, in_maps, core_ids=list(range(NCORES)),
                                trace=trace, tmpdir=tmpdir)


def _canary_ok(out, x, wq_w, wq_b, wk_w, wk_b, wv_w, wv_b):
    """Cheap exact check of causal rows t=0,1 (closed-form, tiny host cost).

    Catches catastrophic HW-mode failures (zeros/garbage) while passing
    reduced-precision rounding. Row 0 attends only key 0 -> out = v[0];
    row 1 is a two-term softmax.
    """
    x2 = np.asarray(x, np.float32)[:, 0:2, :].astype(np.float64)      # [B,2,D]
    q = x2 @ np.asarray(wq_w, np.float64).T + np.asarray(wq_b, np.float64)
    k = x2 @ np.asarray(wk_w, np.float64).T + np.asarray(wk_b, np.float64)
    v = x2 @ np.asarray(wv_w, np.float64).T + np.asarray(wv_b, np.float64)
    exp0 = v[:, 0, :]                                                 # [B,HS]
    s = np.einsum("bh,bsh->bs", q[:, 1, :], k) / np.sqrt(HS)          # [B,2]
    w = np.exp(s - s.max(-1, keepdims=True))
    w = w / w.sum(-1, keepdims=True)
    exp1 = np.einsum("bs,bsh->bh", w, v)
    got = np.stack([out[:, 0, :], out[:, 1, :]], axis=1)
    want = np.stack([exp0, exp1], axis=1)
    rel = np.abs(got - want) / max(np.abs(want).max(), 1e-6)
    return np.isfinite(got).all() and rel.max() < 3e-2


def kernel(x, wq_w, wq_b, wk_w, wk_b, wv_w, wv_b):
    args = (x, wq_w, wq_b, wk_w, wk_b, wv_w, wv_b)
    res = run(make_in_maps(*args, mode=MM_MODE), mode=MM_MODE)
    out = np.stack([np.asarray(res.results[b]["out"], np.float32)
                    for b in range(B)], axis=0)
    if MM_MODE != FALLBACK_MODE and not _canary_ok(out, *args):
        # fast matmul mode produced bad numerics on this HW; fall back to
        # the plain-fp32 kernel
        res = run(make_in_maps(*args, mode=FALLBACK_MODE), mode=FALLBACK_MODE)
        out = np.stack([np.asarray(res.results[b]["out"], np.float32)
                        for b in range(B)], axis=0)
    return out
